# revision 23
# baseline (speedup 1.0000x reference)
"""Bahdanau attention on 8 Trainium2 NeuronCores (Bass/Tile).

Data-parallel over the batch dim (64 -> 8 per core), weights replicated.

Per-core device kernel, for each (batch b, s-tile of 512):
  1. DMA encT tile [e=128 x4, s=512] (host supplies enc pre-transposed
     to [b, e, s] so every DMA is wide and contiguous).
  2. k^T psum [h=128 x4, s=512] = Wk-block^T @ encT-chunk  (16 fp32r MMs)
  3. energy = tanh(k^T + (q[b]+bk) per-partition bias)      (4 ScalarE ops)
  4. scores psum [1, s] = Wv-chunk^T @ energy               (4 fp32r MMs)
  5. p = exp(scores + bv)                                   (1 ScalarE op)
  6. p_m = p * mask, l += sum(p_m)    (1 fused DVE tensor_tensor_reduce)
  7. p_bcast psum [128, s] = ones^T @ p_m                   (1 fp32 MM)
  8. ctx[e] += sum_s encT*p_bcast     (4 fused DVE tensor_tensor_reduce)
Softmax uses no max-subtraction: |scores| <= sqrt(512)+|bv| ~ 23, so
exp() cannot overflow fp32, and masked lanes get exp(-1e10)*... -> 0 via
the mask multiply. Normalization by 1/l happens once per batch at the end.
"""

import numpy as np

import concourse.bacc as bacc
import concourse.mybir as mybir
import concourse.tile as tile
from concourse.bass_utils import run_bass_kernel_spmd
from concourse.dve_ops import TENSOR_TENSOR_REDUCE as TTR_OP

N_CORES = 8
B, S, H, E = 64, 4096, 512, 512
BL = B // N_CORES          # batches per core
ST = 512                   # s-tile size
NT = S // ST               # s-tiles per batch
P = 128                    # partitions
HC = H // P                # h chunks
EC = E // P                # e chunks

F32 = mybir.dt.float32
F32R = mybir.dt.float32r
I32 = mybir.dt.int32
AF = mybir.ActivationFunctionType
ALU = mybir.AluOpType


def _r(ap):
    return ap.bitcast(F32R)


def build_nc(reps=1):
    nc = bacc.Bacc("TRN2", target_bir_lowering=False, debug=False)

    enc_t = nc.dram_tensor("enc_t", [BL, E, S], F32R, kind="ExternalInput")
    hidden = nc.dram_tensor("hidden", [BL, H], F32, kind="ExternalInput")
    mask = nc.dram_tensor("mask", [BL, S], I32, kind="ExternalInput")
    Wq = nc.dram_tensor("Wq", [H, H], F32, kind="ExternalInput")
    bq = nc.dram_tensor("bq", [H], F32, kind="ExternalInput")
    Wk = nc.dram_tensor("Wk", [E, H], F32, kind="ExternalInput")
    bk = nc.dram_tensor("bk", [H], F32, kind="ExternalInput")
    Wv = nc.dram_tensor("Wv", [H], F32, kind="ExternalInput")
    bv = nc.dram_tensor("bv", [1], F32, kind="ExternalInput")
    ctx_out = nc.dram_tensor("ctx_out", [BL, E], F32, kind="ExternalOutput")
    attn_out = nc.dram_tensor("attn_out", [BL, S], F32, kind="ExternalOutput")

    with tile.TileContext(nc) as tc:
        _body(nc, tc, enc_t, hidden, mask, Wq, bq, Wk, bk, Wv, bv,
              ctx_out, attn_out, reps=reps)
    nc.compile()
    return nc


def _body(nc, tc, enc_t, hidden, mask, Wq, bq, Wk, bk, Wv, bv,
          ctx_out, attn_out, reps=1):
    cpool_ctx = tc.tile_pool(name="const_pool", bufs=1)
    cpool = cpool_ctx.__enter__()

    def ctile(shape, dtype, name):
        return cpool.tile(shape, dtype, name=name)

    # ---- persistent constants / weights --------------------------------
    wk_sb = ctile([P, EC, H], F32R, "wk_sb")     # [p, ec, h]
    nc.gpsimd.dma_start(wk_sb[:], Wk.ap().rearrange("(ec p) h -> p ec h", p=P))
    wv_sb = ctile([P, HC], F32R, "wv_sb")        # [p, hc]
    nc.gpsimd.dma_start(wv_sb[:], Wv.ap().rearrange("(hc p) -> p hc", p=P))
    bv_sb = ctile([1, 1], F32, "bv_sb")
    nc.sync.dma_start(bv_sb[:], bv.ap().rearrange("(y x) -> y x", y=1))
    ones_sb = ctile([1, P], F32, "ones_sb")
    nc.vector.memset(ones_sb[:], 1.0)
    ones_r = ctile([1, P], F32R, "ones_r")
    nc.gpsimd.dma_start(ones_r[:], ones_sb[:])

    qt_sb = ctile([P, HC, BL], F32, "qt_sb")    # [h%128, hc, b]
    maskb_f8 = ctile([BL, S], F32, "maskb_f8")  # (mask-1)*1e10 per [b, s]
    ctx_all = ctile([P, BL, EC], F32, "ctx_all")  # [e%128, b, ec]

    # q^T = (hidden @ Wq + bq + bk); setup-only tensors live in a pool
    # released before the main loop.
    with (
        tc.tile_pool(name="setup_sb", bufs=1) as ssb,
        tc.tile_pool(name="setup_ps", bufs=1, space="PSUM") as sps,
    ):
        wq_sb = ssb.tile([P, HC, H], F32, name="wq_sb")
        nc.sync.dma_start(wq_sb[:],
                          Wq.ap().rearrange("(j p) h -> p j h", p=P))
        ht_sb = ssb.tile([P, HC, BL], F32, name="ht_sb")  # hidden^T
        for j in range(HC):
            nc.sync.dma_start(
                ht_sb[:, j, :],
                hidden.ap()[:, j * P:(j + 1) * P].rearrange("b p -> p b"))
        bqk = ssb.tile([P, HC], F32, name="bqk")
        bq_sb = ssb.tile([P, HC], F32, name="bq_sb")
        bk_sb = ssb.tile([P, HC], F32, name="bk_sb")
        nc.sync.dma_start(bq_sb[:], bq.ap().rearrange("(hc p) -> p hc", p=P))
        nc.sync.dma_start(bk_sb[:], bk.ap().rearrange("(hc p) -> p hc", p=P))
        nc.vector.tensor_tensor(bqk[:], bq_sb[:], bk_sb[:], ALU.add)

        qt_ps = sps.tile([P, HC, BL], F32)
        for hc in range(HC):
            for j in range(HC):
                nc.tensor.matmul(
                    qt_ps[:, hc, :],
                    wq_sb[:, j, hc * P:(hc + 1) * P],
                    ht_sb[:, j, :],
                    start=(j == 0), stop=(j == HC - 1),
                )
        for hc in range(HC):
            nc.vector.tensor_scalar_add(
                qt_sb[:, hc, :], qt_ps[:, hc, :], bqk[:, hc:hc + 1])

        mask_i = ssb.tile([BL, S], I32, name="mask_i")
        nc.sync.dma_start(mask_i[:], mask.ap())
        nc.vector.tensor_copy(maskb_f8[:], mask_i[:])
        nc.vector.tensor_scalar(maskb_f8[:], maskb_f8[:], 1.0, 1e10,
                                ALU.subtract, ALU.mult)

    # ---- pools ----------------------------------------------------------
    with (
        tc.tile_pool(name="enc_pool", bufs=3) as enc_pool,
        tc.tile_pool(name="en_pool", bufs=2) as en_pool,
        tc.tile_pool(name="small_pool", bufs=2) as small_pool,
        tc.tile_pool(name="batch_pool", bufs=2) as batch_pool,
        tc.tile_pool(name="scratch_pool", bufs=2) as scratch_pool,
        tc.tile_pool(name="kps_pool", bufs=4, space="PSUM") as kps_pool,
        tc.tile_pool(name="sc_pool", bufs=1, space="PSUM") as sc_pool,
        tc.tile_pool(name="pb_pool", bufs=1, space="PSUM") as pb_pool,
        tc.tile_pool(name="rl_pool", bufs=1, space="PSUM") as rl_pool,
    ):
      for _rep in range(reps):
        for b in range(BL):
            maskb_b = batch_pool.tile([1, S], F32, name="maskb_b")
            nc.sync.dma_start(maskb_b[:], maskb_f8[b:b + 1, :])
            p_all = batch_pool.tile([1, S], F32R, name="p_all")
            l_parts = batch_pool.tile([1, NT], F32, name="l_parts")
            l_acc = batch_pool.tile([1, 1], F32, name="l_acc")

            for t in range(NT):
                s0 = t * ST
                enc_sb = enc_pool.tile([P, EC, ST], F32R, name="enc_sb")
                nc.sync.dma_start(
                    enc_sb[:],
                    enc_t.ap()[b, :, s0:s0 + ST].rearrange(
                        "(ec p) s -> p ec s", p=P),
                )

                energy = en_pool.tile([P, HC, ST], F32R, name="energy")
                for hc in range(HC):
                    kps = kps_pool.tile([P, ST], F32, name="kps")
                    for ec in range(EC):
                        nc.tensor.matmul(
                            kps[:],
                            wk_sb[:, ec, hc * P:(hc + 1) * P],
                            enc_sb[:, ec, :],
                            start=(ec == 0), stop=(ec == EC - 1),
                        )
                    nc.scalar.activation(
                        energy[:, hc, :], kps[:], AF.Tanh,
                        bias=qt_sb[:, hc, b:b + 1], scale=1.0)

                sc_ps = sc_pool.tile([1, ST], F32, name="sc_ps")
                for hc in range(HC):
                    nc.tensor.matmul(
                        sc_ps[:],
                        wv_sb[:, hc:hc + 1],
                        energy[:, hc, :],
                        start=(hc == 0), stop=(hc == HC - 1),
                    )

                scores_m = small_pool.tile([1, ST], F32, name="scores_m")
                nc.vector.tensor_tensor(
                    scores_m[:], sc_ps[:], maskb_b[:, s0:s0 + ST], ALU.add)
                # p (fp32r-rounded) and its tile-partial sum in one ACT op
                nc.scalar.activation(
                    p_all[:, s0:s0 + ST], scores_m[:], AF.Exp,
                    bias=bv_sb[0:1, 0:1], scale=1.0,
                    accum_out=l_parts[:, t:t + 1])

                # broadcast p to 128 partitions (fp32r matmul, 512 cycles)
                pb_ps = pb_pool.tile([P, ST], F32, name="pb_ps")
                nc.tensor.matmul(
                    pb_ps[:], ones_r[:], p_all[:, s0:s0 + ST],
                    start=True, stop=True)

                # ctx[e] accumulation, one fused op per e-chunk
                for ec in range(EC):
                    scr = scratch_pool.tile([P, ST], F32, name="scr")
                    nc.vector._custom_dve(
                        TTR_OP,
                        out=scr[:],
                        in0=enc_sb[:, ec, :].bitcast(F32),
                        in1=pb_ps[:],
                        s0=0.0 if t == 0 else ctx_all[:, b, ec:ec + 1],
                        s1=1.0,
                        accum_out=ctx_all[:, b, ec:ec + 1],
                    )

            # ---- batch epilogue: normalize ------------------------------
            nc.vector.tensor_reduce(
                l_acc[:], l_parts[:], op=ALU.add, axis=mybir.AxisListType.X)
            rl_sb = small_pool.tile([1, 1], F32, name="rl_sb")
            nc.vector.reciprocal(rl_sb[:], l_acc[:])
            attn_sb = batch_pool.tile([1, S], F32, name="attn_sb")
            nc.vector.tensor_scalar_mul(attn_sb[:], p_all[:].bitcast(F32),
                                        rl_sb[0:1, 0:1])
            nc.sync.dma_start(attn_out.ap()[b:b + 1, :], attn_sb[:])

            rl_ps = rl_pool.tile([P, 1], F32, name="rl_ps")
            nc.tensor.matmul(rl_ps[:], ones_sb[:], rl_sb[:],
                             start=True, stop=True)
            nc.vector.tensor_scalar_mul(
                ctx_all[:, b, :], ctx_all[:, b, :], rl_ps[:, 0:1])

        nc.sync.dma_start(
            ctx_out.ap().rearrange("b (ec p) -> p b ec", p=P), ctx_all[:])
    cpool_ctx.__exit__(None, None, None)


_NC = None


def _get_nc():
    global _NC
    if _NC is None:
        _NC = build_nc()
    return _NC


def _make_runner(nc):
    """Build a stable jitted 8-core runner (mirrors bass2jax.run_bass_via_pjrt)
    that accepts device-resident inputs, for repeat-timing. With chain=N the
    kernel body executes N times sequentially inside one dispatch (iterations
    data-chained by feeding ctx_out back as `hidden`), so per-call wall time
    is dispatch_overhead + N * hw_time and the slope isolates hw_time."""
    import jax
    import jax.numpy as jnp
    from jax.sharding import Mesh, PartitionSpec, NamedSharding
    from jax.experimental.shard_map import shard_map
    from concourse import bass2jax, mybir as _mybir

    bass2jax.install_neuronx_cc_hook()
    partition_name = (nc.partition_id_tensor.name
                      if nc.partition_id_tensor else None)
    in_names, out_names, out_avals = [], [], []
    for alloc in nc.m.functions[0].allocations:
        if not isinstance(_mybir.MemoryLocationSet, type) or not isinstance(
                alloc, _mybir.MemoryLocationSet):
            continue
        name = alloc.memorylocations[0].name
        if alloc.kind == "ExternalInput":
            if name != partition_name:
                in_names.append(name)
        elif alloc.kind == "ExternalOutput":
            out_names.append(name)
            out_avals.append(jax.core.ShapedArray(
                tuple(alloc.tensor_shape), _mybir.dt.np(alloc.dtype)))
    n_params = len(in_names)
    all_in_names = in_names + out_names
    if partition_name is not None:
        all_in_names = all_in_names + [partition_name]

    def _exec_once(operands):
        ops = list(operands)
        if partition_name is not None:
            ops.append(bass2jax.partition_id_tensor())
        return bass2jax._bass_exec_p.bind(
            *ops,
            out_avals=tuple(out_avals),
            in_names=tuple(all_in_names),
            out_names=tuple(out_names),
            lowering_input_output_aliases=(),
            sim_require_finite=True,
            sim_require_nnan=True,
            nc=nc,
        )

    def _bdy(*args):
        return tuple(_exec_once(args))

    devices = jax.devices()[:N_CORES]
    mesh = Mesh(np.asarray(devices), ("core",))
    spec = PartitionSpec("core")
    sharded = jax.jit(
        shard_map(_bdy, mesh=mesh,
                  in_specs=(spec,) * (n_params + len(out_names)),
                  out_specs=(spec,) * len(out_names), check_rep=False),
        keep_unused=True,
    )
    sharding = NamedSharding(mesh, spec)
    return sharded, in_names, out_names, out_avals, sharding


def time_hw(inputs, iters=24, reps=32):
    """HW time per kernel execution: paired slope between a 1-rep NEFF and
    an N-rep NEFF (kernel body unrolled N times inside one NEFF). Calls are
    interleaved and the 25th-percentile paired delta is used, which rejects
    the axon dispatch jitter."""
    import time as _time
    import jax

    in_maps = _shard_inputs(**_preprocess(**inputs))
    c1, o1 = _make_caller(_get_nc(), in_maps)
    cn, on = _make_caller(build_nc(reps=reps), in_maps)
    # sanity: rep-unrolled NEFF must produce identical outputs
    for a, b in zip(o1(), on()):
        assert np.array_equal(np.asarray(a), np.asarray(b)), "reps mismatch"
    diffs = []
    for _ in range(iters):
        t0 = _time.perf_counter(); c1(); t1 = _time.perf_counter()
        cn(); t2 = _time.perf_counter()
        diffs.append((t2 - t1) - (t1 - t0))
    diffs.sort()
    q25 = diffs[len(diffs) // 4]
    return max(q25, 0.0) / (reps - 1) * 1e9


def _make_caller(nc, in_maps):
    import jax

    runner, in_names, out_names, out_avals, sharding = _make_runner(nc)
    ci = [jax.device_put(
        np.concatenate([np.asarray(in_maps[c][nm]) for c in range(N_CORES)],
                       axis=0), sharding) for nm in in_names]
    z = [jax.device_put(
        np.zeros((N_CORES * a.shape[0], *a.shape[1:]), a.dtype), sharding)
        for a in out_avals]
    state = {}

    def call():
        out = runner(*ci, *z)
        jax.block_until_ready(out)
        state["out"] = out

    def outs():
        call()
        return state["out"]

    call()  # warmup/compile
    return call, outs


_NULL_NC = None


def _null_nc():
    global _NULL_NC
    if _NULL_NC is None:
        import concourse.tile as _tile
        nc = bacc.Bacc("TRN2", target_bir_lowering=False, debug=False)
        x = nc.dram_tensor("x", [1, 1], F32, kind="ExternalInput")
        y = nc.dram_tensor("y", [1, 1], F32, kind="ExternalOutput")
        with _tile.TileContext(nc) as tc:
            with tc.tile_pool(name="p", bufs=1) as pool:
                t = pool.tile([1, 1], F32)
                nc.sync.dma_start(t[:], x.ap())
                nc.sync.dma_start(y.ap(), t[:])
        nc.compile()
        _NULL_NC = nc
    return _NULL_NC


def _timed_run(nc, in_maps, iters):
    import time as _time
    import jax

    runner, in_names, out_names, out_avals, sharding = _make_runner(nc)
    concat_in = [
        jax.device_put(
            np.concatenate([np.asarray(in_maps[c][n]) for c in range(N_CORES)],
                           axis=0), sharding)
        for n in in_names
    ]
    zeros = [
        jax.device_put(np.zeros((N_CORES * a.shape[0], *a.shape[1:]), a.dtype),
                       sharding)
        for a in out_avals
    ]
    # warmup (compile + first exec)
    out = runner(*concat_in, *zeros)
    jax.block_until_ready(out)
    times = []
    for _ in range(iters):
        t0 = _time.perf_counter()
        out = runner(*concat_in, *zeros)
        jax.block_until_ready(out)
        times.append(_time.perf_counter() - t0)
    return min(times)


def _rne12(a):
    """Round fp32 to the fp32r grid (RNE on the low 12 mantissa bits) --
    bit-exact match of the on-device fp32->fp32r cast, verified on HW."""
    u = a.view(np.uint32).astype(np.uint64)
    r = (u + 0x7FF + ((u >> 12) & 1)) & ~np.uint64(0xFFF)
    return r.astype(np.uint32).view(np.float32)


def _preprocess(hidden, encoder_outputs, mask, Wq, bq, Wk, bk, Wv, bv):
    enc = np.ascontiguousarray(np.asarray(encoder_outputs, dtype=np.float32))
    return dict(
        hidden=np.ascontiguousarray(np.asarray(hidden, dtype=np.float32)),
        # layout transform + fp32r grid rounding (device would do the same
        # rounding in its fp32->fp32r cast; doing it host-side lets the
        # loads go through HWDGE instead of the single SWDGE queue)
        enc_t=np.ascontiguousarray(_rne12(enc).transpose(0, 2, 1)),
        mask=np.ascontiguousarray(np.asarray(mask, dtype=np.int32)),
        Wq=np.ascontiguousarray(np.asarray(Wq, dtype=np.float32)),
        bq=np.ascontiguousarray(np.asarray(bq, dtype=np.float32)),
        Wk=np.ascontiguousarray(np.asarray(Wk, dtype=np.float32)),
        bk=np.ascontiguousarray(np.asarray(bk, dtype=np.float32)),
        Wv=np.ascontiguousarray(np.asarray(Wv, dtype=np.float32).reshape(H)),
        bv=np.ascontiguousarray(np.asarray(bv, dtype=np.float32).reshape(1)),
    )


def _shard_inputs(enc_t, hidden, mask, Wq, bq, Wk, bk, Wv, bv):
    in_maps = []
    for c in range(N_CORES):
        sl = slice(c * BL, (c + 1) * BL)
        in_maps.append({
            "enc_t": enc_t[sl],
            "hidden": hidden[sl],
            "mask": mask[sl],
            "Wq": Wq, "bq": bq, "Wk": Wk, "bk": bk, "Wv": Wv, "bv": bv,
        })
    return in_maps


def kernel(hidden, encoder_outputs, mask, Wq, bq, Wk, bk, Wv, bv):
    pre = _preprocess(hidden, encoder_outputs, mask, Wq, bq, Wk, bk, Wv, bv)
    in_maps = _shard_inputs(**pre)
    nc = _get_nc()
    res = run_bass_kernel_spmd(nc, in_maps, core_ids=list(range(N_CORES)))
    context = np.concatenate([r["ctx_out"] for r in res.results], axis=0)
    attn = np.concatenate([r["attn_out"] for r in res.results], axis=0)
    return context.astype(np.float32), attn.astype(np.float32)


if __name__ == "__main__":
    rng = np.random.default_rng(0)
    inputs = {
        "hidden": rng.standard_normal((B, H), dtype=np.float32),
        "encoder_outputs": rng.standard_normal((B, S, E), dtype=np.float32),
        "mask": rng.integers(0, 2, size=(B, S)).astype(np.int32),
        "Wq": rng.uniform(-0.04, 0.04, (H, H)).astype(np.float32),
        "bq": rng.uniform(-0.04, 0.04, H).astype(np.float32),
        "Wk": rng.uniform(-0.04, 0.04, (E, H)).astype(np.float32),
        "bk": rng.uniform(-0.04, 0.04, H).astype(np.float32),
        "Wv": rng.uniform(-0.04, 0.04, (H, 1)).astype(np.float32),
        "bv": rng.uniform(-0.04, 0.04, 1).astype(np.float32),
    }
    ctx, attn = kernel(**inputs)
    print(ctx.shape, attn.shape, np.abs(ctx).mean(), attn.sum(axis=1)[:4])
